# revision 19
# baseline (speedup 1.0000x reference)
"""Depthwise 3x3 conv over each depth slice of x[B,H,W,D,C] on 8 trn2 cores.

Strategy:
  - Data-parallel over batch: core i handles x[i] ([H,W,D,C] = [64,64,32,64]).
  - Per core, loop over 16 depth-pair groups; partitions = (d_parity, C) = 128,
    free axis = spatial (H*W) so the per-(d,c) tap weights are per-partition
    scalars and each tap is one fused (x*w + acc) instruction.
  - HBM has C contiguous, so the (spatial, channel) <-> (channel, spatial)
    layout change is done on-chip with PE transposes (128x128 blocks).
  - SAME zero padding handled by a 65-stride padded slab with zeroed guard
    rows/pad column so every tap is a flat shifted read.
"""

import os
from contextlib import ExitStack

import numpy as np

import concourse.bass as bass
import concourse.mybir as mybir
import concourse.tile as tile
from concourse.bass_utils import run_bass_kernel_spmd
from concourse.masks import make_identity
from concourse.tile import add_dep_helper

F32 = mybir.dt.float32

B, H, W, D, C = 8, 64, 64, 32, 64
G = D // 2              # 16 depth-pair groups per core
RS = W + 1              # 65: padded row stride (col 64 of each row is zero)
DATA0 = RS + 1          # 66: flat offset of (h=0, w=0) in the slab
SLAB = DATA0 + 64 * RS + RS + 1   # 66 + 4160 + 66 = 4292
CONVL = 64 * RS         # 4160 = span of a [64 rows x 65] view

MULT = mybir.AluOpType.mult
ADD = mybir.AluOpType.add

# Tap execution plan: (dh, dw) -> engine. Seed tap (0,0) runs as a DVE
# tensor_scalar (w*x + b). Remaining 8 are fused scalar_tensor_tensor
# accumulates split DVE/GPSIMD.
SEED_TAP = (0, 0)
OTHER_TAPS = [
    (dh, dw) for dh in (-1, 0, 1) for dw in (-1, 0, 1) if (dh, dw) != SEED_TAP
]
# indices into OTHER_TAPS that run on gpsimd (the rest go on the vector
# engine). The LAST tap must stay on nc.vector: it writes the separate y2
# tile, keeping y2 single-writer/single-engine so the out-transpose absorber
# needs only one sem wait.
GPS_TAPS = set()  # walrus rejects TensorScalarPtr on Pool/GPSIMD


def _build_nc():
    nc = bass.Bass("TRN2", target_bir_lowering=False, debug=False)
    xs = nc.dram_tensor("xs", [H, W, D, C], F32, kind="ExternalInput").ap()
    ws = nc.dram_tensor("ws", [128, G * 9], F32, kind="ExternalInput").ap()
    bs = nc.dram_tensor("bs", [128, G], F32, kind="ExternalInput").ap()
    ys = nc.dram_tensor("ys", [H, W, D, C], F32, kind="ExternalOutput").ap()

    with tile.TileContext(nc) as tc, ExitStack() as ctx:
        consts = ctx.enter_context(tc.tile_pool(name="consts", bufs=1))
        ident = consts.tile([128, 128], F32)
        make_identity(nc, ident[:])
        wst = consts.tile([128, G * 9], F32)
        nc.sync.dma_start(wst[:], ws)
        bst = consts.tile([128, G], F32)
        nc.sync.dma_start(bst[:], bs)

        xdp = ctx.enter_context(tc.tile_pool(name="xd", bufs=2))
        xap = ctx.enter_context(tc.tile_pool(name="xa", bufs=2))
        yp = ctx.enter_context(tc.tile_pool(name="y", bufs=2))
        ydp = ctx.enter_context(tc.tile_pool(name="yd", bufs=2))
        pin = ctx.enter_context(
            tc.tile_pool(name="pin", bufs=2, space=bass.MemorySpace.PSUM)
        )
        pout = ctx.enter_context(
            tc.tile_pool(name="pout", bufs=2, space=bass.MemorySpace.PSUM)
        )
        pdum = ctx.enter_context(
            tc.tile_pool(name="pdum", bufs=1, space=bass.MemorySpace.PSUM)
        )

        # PE instructions accept at most ONE sync wait in this toolchain, so:
        #  - an ACT "toucher" is made the first accessor of every psum tile
        #    (it can carry the multi-engine slot release-set),
        #  - tiny absorber matmuls into a write-only dummy psum tile observe
        #    one semaphore each (DMA / toucher / y2) before the real
        #    transposes, which are pinned behind them with add_dep_helper.
        dummy = pdum.tile([128, 8], F32)

        def pe_absorb(col, dep=None):
            mm = nc.tensor.matmul(
                dummy[0:1, 0:1], col, ident[:, 0:1], skip_group_check=True
            )
            if dep is not None:
                add_dep_helper(mm.ins, dep.ins, reason="observe tick")
            return mm

        pe_absorb(ident[:, 0:1])  # PE observes the identity build once

        for g in range(G):
            # ---- load: [128 spatial, 32 blocks, 128 ch] (512B bursts in HBM)
            src = xs[:, :, 2 * g : 2 * g + 2, :].rearrange(
                "(j ph) w dp c -> (ph w) j (dp c)", ph=2
            )
            xd = xdp.tile([128, 32, 128], F32, tag="xd")
            nc.sync.dma_start(xd[:], src)

            # ---- padded slab (channel-major)
            xa = xap.tile([128, SLAB], F32, tag="xa")
            nc.vector.memset(xa[:, 0:DATA0], 0.0)
            nc.vector.memset(xa[:, DATA0 + 63 * RS + 64 : SLAB], 0.0)
            padcol = xa[:, DATA0 + 64 : DATA0 + 64 + CONVL].rearrange(
                "p (r o) -> p r o", o=RS
            )[:, :, 0:1]
            nc.vector.memset(padcol, 0.0)

            absA = pe_absorb(xd[:, 0, 0:1])  # PE observes xd's DMA
            for q in range(4):
                pt = pin.tile([128, 1024], F32, tag="pin")
                touch = pt[0:1, :].rearrange("p (j c) -> p j c", j=8)[:, :, 0:1]
                tch = nc.scalar.copy(
                    touch, ident[0:1, 0:8].rearrange("p (j c) -> p j c", c=1)
                )
                absB = pe_absorb(ident[:, 0:1], dep=tch)
                for jo in range(8):
                    j = 8 * q + jo
                    t = nc.tensor.transpose(
                        pt[:, 128 * jo : 128 * (jo + 1)], xd[:, j, :], ident[:]
                    )
                    add_dep_helper(t.ins, absB.ins, reason="after toucher-obs")
                    add_dep_helper(t.ins, absA.ins, reason="after dma-obs")
                dst = xa[:, DATA0 + 1040 * q : DATA0 + 1040 * q + 1040].rearrange(
                    "p (j r b) -> p j r b", j=8, b=RS
                )[:, :, :, 0:64]
                srcp = pt[:].rearrange("p (j r b) -> p j r b", j=8, b=64)
                nc.scalar.copy(dst, srcp)

            # ---- conv: y[h,w] = b + sum_t w_t * x[h+dh, w+dw]
            y = yp.tile([128, 4096], F32, tag="y")
            yv = y[:].rearrange("p (a b) -> p a b", b=64)

            def xsh(dh, dw, xa=xa):
                s0 = DATA0 + dh * RS + dw
                return xa[:, s0 : s0 + CONVL].rearrange("p (a b) -> p a b", b=RS)[
                    :, :, 0:64
                ]

            def wap(dh, dw, g=g):
                i = g * 9 + (dh + 1) * 3 + (dw + 1)
                return wst[:, i : i + 1]

            y2 = yp.tile([128, 4096], F32, tag="y2")
            y2v = y2[:].rearrange("p (a b) -> p a b", b=64)

            nc.vector.tensor_scalar(
                yv, xsh(*SEED_TAP), wap(*SEED_TAP), bst[:, g : g + 1], MULT, ADD
            )
            for i, (dh, dw) in enumerate(OTHER_TAPS):
                eng = nc.gpsimd if i in GPS_TAPS else nc.vector
                out = y2v if i == len(OTHER_TAPS) - 1 else yv
                eng.scalar_tensor_tensor(out, xsh(dh, dw), wap(dh, dw), yv, MULT, ADD)

            # ---- transpose back + store
            yd = ydp.tile([128, 32, 128], F32, tag="yd")
            absC = pe_absorb(y2[:, 0:1])  # PE observes y2's final writer
            for q in range(8):
                pt = pout.tile([128, 512], F32, tag="pout")
                touch = pt[0:1, :].rearrange("p (j c) -> p j c", j=4)[:, :, 0:1]
                tch = nc.scalar.copy(
                    touch, ident[0:1, 0:4].rearrange("p (j c) -> p j c", c=1)
                )
                absB = pe_absorb(ident[:, 0:1], dep=tch)
                for jo in range(4):
                    j = 4 * q + jo
                    t = nc.tensor.transpose(
                        pt[:, 128 * jo : 128 * (jo + 1)],
                        y2[:, 128 * j : 128 * (j + 1)],
                        ident[:],
                    )
                    add_dep_helper(t.ins, absB.ins, reason="after toucher-obs")
                    add_dep_helper(t.ins, absC.ins, reason="after y2-obs")
                nc.scalar.copy(
                    yd[:, 4 * q : 4 * q + 4, :],
                    pt[:].rearrange("p (j c) -> p j c", j=4),
                )
            dst = ys[:, :, 2 * g : 2 * g + 2, :].rearrange(
                "(j ph) w dp c -> (ph w) j (dp c)", ph=2
            )
            nc.sync.dma_start(dst, yd[:])

    return nc


# walrus setupSyncWait caps per engine struct: PE Matmult takes 1 sem wait,
# ACT/DVE/Pool compute ops take 2. Tile sometimes attaches more (psum slot
# release-sets). Hoist the excess onto injected same-engine Drains (Tile's
# own epilogue Drain carries 12 waits, so Drain accepts many).
_WAIT_CAPS = {"PE": 1, "Activation": 1, "DVE": 1, "Pool": 1, "SP": 1}
_SPLIT_SEQ = [0]


def _split_waits(nc):
    fn = nc.m.functions[0]
    nsplit = 0
    for blk in fn.blocks:
        out = []
        changed = False
        for ins in blk.instructions:
            si = ins.sync_info
            waits = list(si.on_wait) if si is not None and si.on_wait else []
            eng = getattr(ins, "engine", None)
            engname = getattr(eng, "value", None) or str(eng)
            cap = _WAIT_CAPS.get(engname)
            if cap is not None and len(waits) > cap:
                excess, keep = waits[:-cap], waits[-cap:]
                for w in excess:
                    _SPLIT_SEQ[0] += 1
                    d = mybir.InstDrain(name=f"I-ws{_SPLIT_SEQ[0]}", ins=[], outs=[])
                    d.engine = eng
                    d.sync_info = mybir.SyncInfo(on_wait=[w], on_update=[])
                    out.append(d)
                ins.sync_info = mybir.SyncInfo(
                    on_wait=keep, on_update=list(si.on_update or [])
                )
                changed = True
                nsplit += 1
            out.append(ins)
        if changed:
            blk.instructions = out
    return nsplit


_NC_CACHE = None


def _get_nc():
    global _NC_CACHE
    if _NC_CACHE is None:
        nc = _build_nc()
        _split_waits(nc)
        _NC_CACHE = nc
    return _NC_CACHE


class Runner:
    """Persistent PJRT executor for an SPMD bass module (axon path).

    Mirrors bass2jax.run_bass_via_pjrt's multi-core branch but keeps the
    jitted callable so repeated (timed) invocations don't recompile.
    """

    def __init__(self, nc, n_cores=8):
        import jax
        from jax.experimental.shard_map import shard_map
        from jax.sharding import Mesh, PartitionSpec
        from concourse import bass2jax

        bass2jax.install_neuronx_cc_hook()
        self.jax = jax
        self.nc = nc
        self.n = n_cores
        partition_name = (
            nc.partition_id_tensor.name if nc.partition_id_tensor else None
        )
        in_names, out_names, out_avals = [], [], []
        for alloc in nc.m.functions[0].allocations:
            if not isinstance(alloc, mybir.MemoryLocationSet):
                continue
            name = alloc.memorylocations[0].name
            if alloc.kind == "ExternalInput":
                if name != partition_name:
                    in_names.append(name)
            elif alloc.kind == "ExternalOutput":
                out_names.append(name)
                out_avals.append(
                    jax.core.ShapedArray(
                        tuple(alloc.tensor_shape), mybir.dt.np(alloc.dtype)
                    )
                )
        self.in_names = list(in_names)
        self.out_names = out_names
        self.out_avals = out_avals
        bind_in_names = list(in_names) + list(out_names)
        if partition_name is not None:
            bind_in_names.append(partition_name)
        bind_in_names = tuple(bind_in_names)
        n_params = len(in_names)
        n_outs = len(out_names)

        def _body(*args):
            operands = list(args)
            if partition_name is not None:
                operands.append(bass2jax.partition_id_tensor())
            outs = bass2jax._bass_exec_p.bind(
                *operands,
                out_avals=tuple(out_avals),
                in_names=bind_in_names,
                out_names=tuple(out_names),
                lowering_input_output_aliases=(),
                sim_require_finite=True,
                sim_require_nnan=True,
                nc=nc,
            )
            return tuple(outs)

        devices = jax.devices()[:n_cores]
        self.mesh = Mesh(np.asarray(devices), ("core",))
        self.spec = PartitionSpec("core")
        in_specs = (self.spec,) * (n_params + n_outs)
        out_specs = (self.spec,) * n_outs
        donate = tuple(range(n_params, n_params + n_outs))
        self.fn = jax.jit(
            shard_map(
                _body,
                mesh=self.mesh,
                in_specs=in_specs,
                out_specs=out_specs,
                check_rep=False,
            ),
            donate_argnums=donate,
            keep_unused=True,
        )
        sharding = jax.sharding.NamedSharding(self.mesh, self.spec)
        self.zeros_fn = jax.jit(
            lambda: tuple(
                self.jax.numpy.zeros((n_cores * a.shape[0], *a.shape[1:]), a.dtype)
                for a in out_avals
            ),
            out_shardings=(sharding,) * n_outs,
        )

    def put_inputs(self, in_maps):
        """in_maps: per-core dict name->np.ndarray. Returns device arrays."""
        jax = self.jax
        sharding = jax.sharding.NamedSharding(self.mesh, self.spec)
        arrs = []
        for name in self.in_names:
            cat = np.concatenate([np.asarray(m[name]) for m in in_maps], axis=0)
            arrs.append(jax.device_put(cat, sharding))
        jax.block_until_ready(arrs)
        return arrs

    def __call__(self, dev_inputs):
        zs = self.zeros_fn()
        self.jax.block_until_ready(zs)
        out = self.fn(*dev_inputs, *zs)
        self.jax.block_until_ready(out)
        return out

    def time_it(self, dev_inputs, reps=10):
        import time as _t

        ts = []
        for _ in range(reps):
            zs = self.zeros_fn()
            self.jax.block_until_ready(zs)
            t0 = _t.perf_counter()
            out = self.fn(*dev_inputs, *zs)
            self.jax.block_until_ready(out)
            ts.append(_t.perf_counter() - t0)
        return ts

    def to_numpy(self, out):
        n = self.n
        return [
            {
                name: np.asarray(out[i]).reshape(n, *self.out_avals[i].shape)[c]
                for i, name in enumerate(self.out_names)
            }
            for c in range(n)
        ]


_RUNNER = None


def _get_runner():
    global _RUNNER
    if _RUNNER is None:
        _RUNNER = Runner(_get_nc(), B)
    return _RUNNER


def _prep_wb(w, b):
    # ws[p, g*9 + kh*3 + kw] = w[2g + p//64, kh, kw, p%64]
    w = np.asarray(w, dtype=np.float32).reshape(G, 2, 9, C)  # (g, dp, tap, c)
    ws = np.ascontiguousarray(w.transpose(1, 3, 0, 2).reshape(128, G * 9))
    b = np.asarray(b, dtype=np.float32).reshape(G, 2, C)
    bs = np.ascontiguousarray(b.transpose(1, 2, 0).reshape(128, G))
    return ws, bs


def _in_maps(inputs):
    x = np.asarray(inputs["x"], dtype=np.float32)
    ws, bs = _prep_wb(inputs["w"], inputs["b"])
    return [{"xs": np.ascontiguousarray(x[i]), "ws": ws, "bs": bs} for i in range(B)]


def kernel(**inputs) -> np.ndarray:
    r = _get_runner()
    dev = r.put_inputs(_in_maps(inputs))
    res = r.to_numpy(r(dev))
    return np.stack([m["ys"] for m in res], axis=0)


# revision 31
# speedup vs baseline: 1.2473x; 1.2473x over previous
"""Depthwise 3x3 conv over each depth slice of x[B,H,W,D,C] on 8 trn2 cores.

Strategy:
  - Data-parallel over batch: core i handles x[i] ([H,W,D,C] = [64,64,32,64]).
  - Per core, loop over 16 depth-pair groups; partitions = (d_parity, C) = 128,
    free axis = spatial (H*W) so the per-(d,c) tap weights are per-partition
    scalars and each tap is one fused (x*w + acc) instruction.
  - HBM has C contiguous, so the (spatial, channel) <-> (channel, spatial)
    layout change is done on-chip with PE transposes (128x128 blocks).
  - SAME zero padding handled by a 65-stride padded slab with zeroed guard
    rows/pad column so every tap is a flat shifted read.
"""

import os
from contextlib import ExitStack

import numpy as np

import concourse.bass as bass
import concourse.mybir as mybir
import concourse.tile as tile
from concourse.bass_utils import run_bass_kernel_spmd
from concourse.masks import make_identity
from concourse.tile import add_dep_helper

F32 = mybir.dt.float32

B, H, W, D, C = 8, 64, 64, 32, 64
G = D // 2              # 16 depth-pair groups per core
RS = W + 1              # 65: padded row stride (col 64 of each row is zero)
DATA0 = RS + 1          # 66: flat offset of (h=0, w=0) in the slab
SLAB = DATA0 + 64 * RS + RS + 1   # 66 + 4160 + 66 = 4292
CONVL = 64 * RS         # 4160 = span of a [64 rows x 65] view

MULT = mybir.AluOpType.mult
ADD = mybir.AluOpType.add

# Tap split: first N_PE_TAPS run as diagonal matmuls on the TensorEngine
# accumulating into PSUM (plus the bias, seeded there too); the rest run as
# fused scalar_tensor_tensor accumulates on the vector engine, whose first
# op reads the PSUM partial as its accumulator input.
ALL_TAPS = [(dh, dw) for dh in (-1, 0, 1) for dw in (-1, 0, 1)]
# PE diag-matmul taps need float32r to stream at full rate, but the BIR
# verifier then requires every producer feeding the matmul to round to
# f32r (including the x slab itself) — unacceptable precision risk, so the
# conv runs entirely on the vector engine (N_PE_TAPS = 0).
N_PE_TAPS = 0
PE_TAPS = ALL_TAPS[:N_PE_TAPS]
DVE_TAPS = ALL_TAPS[N_PE_TAPS:]
# run the 128x128 PE transposes with float32r operands (1.5 vs 2.0
# cycles/row). Identity-matmul data movement; exactness verified on HW.
TRANSPOSE_F32R = False


def _build_nc():
    nc = bass.Bass("TRN2", target_bir_lowering=False, debug=False)
    xs = nc.dram_tensor("xs", [H, W, D, C], F32, kind="ExternalInput").ap()
    ws = nc.dram_tensor("ws", [128, G * 9], F32, kind="ExternalInput").ap()
    bs = nc.dram_tensor("bs", [128, G], F32, kind="ExternalInput").ap()
    ys = nc.dram_tensor("ys", [H, W, D, C], F32, kind="ExternalOutput").ap()

    with tile.TileContext(nc) as tc, ExitStack() as ctx:
        consts = ctx.enter_context(tc.tile_pool(name="consts", bufs=1))
        ident = consts.tile([128, 128], F32)
        make_identity(nc, ident[:])
        ones = consts.tile([128, 512], F32)
        nc.vector.memset(ones[:], 1.0)
        wst = consts.tile([128, G * 9], F32)
        nc.sync.dma_start(wst[:], ws)
        bst = consts.tile([128, G], F32)
        nc.sync.dma_start(bst[:], bs)

        xdp = ctx.enter_context(tc.tile_pool(name="xd", bufs=2))
        xap = ctx.enter_context(tc.tile_pool(name="xa", bufs=2))
        yp = ctx.enter_context(tc.tile_pool(name="y", bufs=2))
        ydp = ctx.enter_context(tc.tile_pool(name="yd", bufs=2))
        dgp = ctx.enter_context(tc.tile_pool(name="diag", bufs=2))
        pin = ctx.enter_context(
            tc.tile_pool(name="pin", bufs=2, space=bass.MemorySpace.PSUM)
        )
        pout = ctx.enter_context(
            tc.tile_pool(name="pout", bufs=2, space=bass.MemorySpace.PSUM)
        )
        pp = ctx.enter_context(
            tc.tile_pool(name="pp", bufs=1, space=bass.MemorySpace.PSUM)
        )
        pdum = ctx.enter_context(
            tc.tile_pool(name="pdum", bufs=1, space=bass.MemorySpace.PSUM)
        )

        # PE instructions accept at most ONE sync wait in this toolchain, so:
        #  - an ACT "toucher" is made the first accessor of every psum tile
        #    (it can carry the multi-engine slot release-set),
        #  - tiny absorber matmuls into a write-only dummy psum tile observe
        #    one semaphore each (DMA / toucher / y2) before the real
        #    transposes, which are pinned behind them with add_dep_helper.
        dummy = pdum.tile([128, 8], F32)

        def pe_absorb(col, dep=None):
            mm = nc.tensor.matmul(
                dummy[0:1, 0:1], col, ident[:, 0:1], skip_group_check=True
            )
            if dep is not None:
                add_dep_helper(mm.ins, dep.ins, reason="observe tick")
            return mm

        pe_absorb(ident[:, 0:1])  # PE observes the identity build once

        for g in range(G):
            # ---- load: [128 spatial, 32 blocks, 128 ch] (512B bursts in HBM)
            src = xs[:, :, 2 * g : 2 * g + 2, :].rearrange(
                "(j ph) w dp c -> (ph w) j (dp c)", ph=2
            )
            xd = xdp.tile([128, 32, 128], F32, tag="xd")
            nc.sync.dma_start(xd[:], src)

            # ---- padded slab (channel-major); pads zeroed on gpsimd (idle)
            xa = xap.tile([128, SLAB], F32, tag="xa")
            nc.gpsimd.memset(xa[:, 0:DATA0], 0.0)
            nc.gpsimd.memset(xa[:, DATA0 + 63 * RS + 64 : SLAB], 0.0)
            padcol = xa[:, DATA0 + 64 : DATA0 + 64 + CONVL].rearrange(
                "p (r o) -> p r o", o=RS
            )[:, :, 0:1]
            nc.gpsimd.memset(padcol, 0.0)

            absA = pe_absorb(xd[:, 0, 0:1])  # PE observes xd's DMA
            last_copy = None
            for q in range(8):
                pt = pin.tile([128, 512], F32, tag="pin")
                touch = pt[0:1, :].rearrange("p (j c) -> p j c", j=4)[:, :, 0:1]
                tch = nc.scalar.copy(
                    touch, ident[0:1, 0:4].rearrange("p (j c) -> p j c", c=1)
                )
                absB = pe_absorb(ident[:, 0:1], dep=tch)
                for jo in range(4):
                    j = 4 * q + jo
                    if TRANSPOSE_F32R:
                        R = mybir.dt.float32r
                        t = nc.tensor.transpose(
                            pt[:, 128 * jo : 128 * (jo + 1)].bitcast(R),
                            xd[:, j, :].bitcast(R),
                            ident[:].bitcast(R),
                        )
                    else:
                        t = nc.tensor.transpose(
                            pt[:, 128 * jo : 128 * (jo + 1)], xd[:, j, :], ident[:]
                        )
                    add_dep_helper(t.ins, absB.ins, reason="after toucher-obs")
                    add_dep_helper(t.ins, absA.ins, reason="after dma-obs")
                dst = xa[:, DATA0 + 520 * q : DATA0 + 520 * q + 520].rearrange(
                    "p (j r b) -> p j r b", j=4, b=RS
                )[:, :, :, 0:64]
                srcp = pt[:].rearrange("p (j r b) -> p j r b", j=4, b=64)
                last_copy = nc.scalar.copy(dst, srcp)

            # ---- conv: y[h,w] = b + sum_t w_t * x[h+dh, w+dw]
            # PE: bias + N_PE_TAPS taps as diag-matmuls accumulating into a
            # PSUM quarter; DVE: remaining taps as fused STT, first one
            # reading the PSUM partial, last one writing y2.
            def wap(dh, dw, g=g):
                i = g * 9 + (dh + 1) * 3 + (dw + 1)
                return wst[:, i : i + 1]

            y = yp.tile([128, 4096], F32, tag="y")
            y2 = yp.tile([128, 4096], F32, tag="y2")

            if not PE_TAPS:
                # all-DVE conv: tensor_scalar seed (w*x + b), then fused
                # scalar_tensor_tensor accumulates; last tap writes y2.
                yv = y[:].rearrange("p (a b) -> p a b", b=64)
                y2v = y2[:].rearrange("p (a b) -> p a b", b=64)

                def xsh(dh, dw, xa=xa):
                    s0 = DATA0 + dh * RS + dw
                    return xa[:, s0 : s0 + CONVL].rearrange(
                        "p (a b) -> p a b", b=RS
                    )[:, :, 0:64]

                (sh, sw), rest = DVE_TAPS[0], DVE_TAPS[1:]
                nc.vector.tensor_scalar(
                    yv, xsh(sh, sw), wap(sh, sw), bst[:, g : g + 1], MULT, ADD
                )
                for i, (dh, dw) in enumerate(rest):
                    out = y2v if i == len(rest) - 1 else yv
                    nc.vector.scalar_tensor_tensor(
                        out, xsh(dh, dw), wap(dh, dw), yv, MULT, ADD
                    )

            diag_b = dgp.tile([128, 128], F32, tag="dbias")
            nc.vector.tensor_scalar(
                diag_b[:], ident[:], bst[:, g : g + 1], None, MULT
            )
            diags = []
            last_diag = None
            for i, (dh, dw) in enumerate(PE_TAPS):
                dt_ = dgp.tile([128, 128], F32, tag=f"d{i}")
                last_diag = nc.vector.tensor_scalar(
                    dt_[:], ident[:], wap(dh, dw), None, MULT
                )
                diags.append(dt_)

            abs_xa = pe_absorb(ident[:, 0:1], dep=last_copy)
            abs_dg = pe_absorb(ident[:, 0:1], dep=last_diag)

            for q in range(4 if PE_TAPS else 0):
                Pq = pp.tile([128, 1024], F32, tag="pp")
                touch = Pq[0:1, :].rearrange("p (h c) -> p h c", h=2)[:, :, 0:1]
                tch = nc.scalar.copy(
                    touch, ident[0:1, 0:2].rearrange("p (h c) -> p h c", c=1)
                )
                absB = pe_absorb(ident[:, 0:1], dep=tch)
                # float32r: same bits as fp32, PE multiplies at reduced
                # precision but streams at 1 cycle/row instead of fp32's 4.
                F32R = mybir.dt.float32r
                for h in range(2):
                    r0 = 16 * q + 8 * h
                    mms = []
                    mm = nc.tensor.matmul(
                        Pq[:, 512 * h : 512 * (h + 1)],
                        diag_b[:].bitcast(F32R),
                        ones[:].bitcast(F32R),
                        start=True,
                        stop=False,
                    )
                    mms.append(mm)
                    for i, (dh, dw) in enumerate(PE_TAPS):
                        o = DATA0 + dh * RS + dw + r0 * RS
                        rhs = xa[:, o : o + 520].rearrange(
                            "p (r b) -> p r b", b=RS
                        )[:, :, 0:64]
                        mm = nc.tensor.matmul(
                            Pq[:, 512 * h : 512 * (h + 1)],
                            diags[i][:].bitcast(F32R),
                            rhs.bitcast(F32R),
                            start=False,
                            stop=(i == len(PE_TAPS) - 1),
                        )
                        mms.append(mm)
                    for mm in mms:
                        add_dep_helper(mm.ins, absB.ins, reason="after toucher")
                        add_dep_helper(mm.ins, abs_xa.ins, reason="after xa")
                        add_dep_helper(mm.ins, abs_dg.ins, reason="after diags")

                yq = y[:, 1024 * q : 1024 * (q + 1)].rearrange(
                    "p (a b) -> p a b", b=64
                )
                y2q = y2[:, 1024 * q : 1024 * (q + 1)].rearrange(
                    "p (a b) -> p a b", b=64
                )
                pv = Pq[:].rearrange("p (a b) -> p a b", b=64)
                for i, (dh, dw) in enumerate(DVE_TAPS):
                    o = DATA0 + dh * RS + dw + 16 * q * RS
                    in0 = xa[:, o : o + 1040].rearrange("p (a b) -> p a b", b=RS)[
                        :, :, 0:64
                    ]
                    in1 = pv if i == 0 else yq
                    out = y2q if i == len(DVE_TAPS) - 1 else yq
                    nc.vector.scalar_tensor_tensor(
                        out, in0, wap(dh, dw), in1, MULT, ADD
                    )

            # ---- transpose back + store
            yd = ydp.tile([128, 32, 128], F32, tag="yd")
            absC = pe_absorb(y2[:, 0:1])  # PE observes y2's final writer
            for q in range(8):
                pt = pout.tile([128, 512], F32, tag="pout")
                touch = pt[0:1, :].rearrange("p (j c) -> p j c", j=4)[:, :, 0:1]
                tch = nc.scalar.copy(
                    touch, ident[0:1, 0:4].rearrange("p (j c) -> p j c", c=1)
                )
                absB = pe_absorb(ident[:, 0:1], dep=tch)
                for jo in range(4):
                    j = 4 * q + jo
                    if TRANSPOSE_F32R:
                        R = mybir.dt.float32r
                        t = nc.tensor.transpose(
                            pt[:, 128 * jo : 128 * (jo + 1)].bitcast(R),
                            y2[:, 128 * j : 128 * (j + 1)].bitcast(R),
                            ident[:].bitcast(R),
                        )
                    else:
                        t = nc.tensor.transpose(
                            pt[:, 128 * jo : 128 * (jo + 1)],
                            y2[:, 128 * j : 128 * (j + 1)],
                            ident[:],
                        )
                    add_dep_helper(t.ins, absB.ins, reason="after toucher-obs")
                    add_dep_helper(t.ins, absC.ins, reason="after y2-obs")
                nc.scalar.copy(
                    yd[:, 4 * q : 4 * q + 4, :],
                    pt[:].rearrange("p (j c) -> p j c", j=4),
                )
            dst = ys[:, :, 2 * g : 2 * g + 2, :].rearrange(
                "(j ph) w dp c -> (ph w) j (dp c)", ph=2
            )
            nc.sync.dma_start(dst, yd[:])

    return nc


# walrus setupSyncWait caps per engine struct: PE Matmult takes 1 sem wait,
# ACT/DVE/Pool compute ops take 2. Tile sometimes attaches more (psum slot
# release-sets). Hoist the excess onto injected same-engine Drains (Tile's
# own epilogue Drain carries 12 waits, so Drain accepts many).
_WAIT_CAPS = {"PE": 1, "Activation": 1, "DVE": 1, "Pool": 1, "SP": 1}
_SPLIT_SEQ = [0]


def _split_waits(nc):
    fn = nc.m.functions[0]
    nsplit = 0
    for blk in fn.blocks:
        out = []
        changed = False
        for ins in blk.instructions:
            si = ins.sync_info
            waits = list(si.on_wait) if si is not None and si.on_wait else []
            eng = getattr(ins, "engine", None)
            engname = getattr(eng, "value", None) or str(eng)
            cap = _WAIT_CAPS.get(engname)
            if cap is not None and len(waits) > cap:
                excess, keep = waits[:-cap], waits[-cap:]
                for w in excess:
                    _SPLIT_SEQ[0] += 1
                    d = mybir.InstDrain(name=f"I-ws{_SPLIT_SEQ[0]}", ins=[], outs=[])
                    d.engine = eng
                    d.sync_info = mybir.SyncInfo(on_wait=[w], on_update=[])
                    out.append(d)
                ins.sync_info = mybir.SyncInfo(
                    on_wait=keep, on_update=list(si.on_update or [])
                )
                changed = True
                nsplit += 1
            out.append(ins)
        if changed:
            blk.instructions = out
    return nsplit


_NC_CACHE = None


def _get_nc():
    global _NC_CACHE
    if _NC_CACHE is None:
        nc = _build_nc()
        _split_waits(nc)
        _NC_CACHE = nc
    return _NC_CACHE


class Runner:
    """Persistent PJRT executor for an SPMD bass module (axon path).

    Mirrors bass2jax.run_bass_via_pjrt's multi-core branch but keeps the
    jitted callable so repeated (timed) invocations don't recompile.
    """

    def __init__(self, nc, n_cores=8):
        import jax
        from jax.experimental.shard_map import shard_map
        from jax.sharding import Mesh, PartitionSpec
        from concourse import bass2jax

        bass2jax.install_neuronx_cc_hook()
        self.jax = jax
        self.nc = nc
        self.n = n_cores
        partition_name = (
            nc.partition_id_tensor.name if nc.partition_id_tensor else None
        )
        in_names, out_names, out_avals = [], [], []
        for alloc in nc.m.functions[0].allocations:
            if not isinstance(alloc, mybir.MemoryLocationSet):
                continue
            name = alloc.memorylocations[0].name
            if alloc.kind == "ExternalInput":
                if name != partition_name:
                    in_names.append(name)
            elif alloc.kind == "ExternalOutput":
                out_names.append(name)
                out_avals.append(
                    jax.core.ShapedArray(
                        tuple(alloc.tensor_shape), mybir.dt.np(alloc.dtype)
                    )
                )
        self.in_names = list(in_names)
        self.out_names = out_names
        self.out_avals = out_avals
        bind_in_names = list(in_names) + list(out_names)
        if partition_name is not None:
            bind_in_names.append(partition_name)
        bind_in_names = tuple(bind_in_names)
        n_params = len(in_names)
        n_outs = len(out_names)

        def _body(*args):
            operands = list(args)
            if partition_name is not None:
                operands.append(bass2jax.partition_id_tensor())
            outs = bass2jax._bass_exec_p.bind(
                *operands,
                out_avals=tuple(out_avals),
                in_names=bind_in_names,
                out_names=tuple(out_names),
                lowering_input_output_aliases=(),
                sim_require_finite=True,
                sim_require_nnan=True,
                nc=nc,
            )
            return tuple(outs)

        devices = jax.devices()[:n_cores]
        self.mesh = Mesh(np.asarray(devices), ("core",))
        self.spec = PartitionSpec("core")
        in_specs = (self.spec,) * (n_params + n_outs)
        out_specs = (self.spec,) * n_outs
        donate = tuple(range(n_params, n_params + n_outs))
        self.fn = jax.jit(
            shard_map(
                _body,
                mesh=self.mesh,
                in_specs=in_specs,
                out_specs=out_specs,
                check_rep=False,
            ),
            donate_argnums=donate,
            keep_unused=True,
        )
        sharding = jax.sharding.NamedSharding(self.mesh, self.spec)
        self.zeros_fn = jax.jit(
            lambda: tuple(
                self.jax.numpy.zeros((n_cores * a.shape[0], *a.shape[1:]), a.dtype)
                for a in out_avals
            ),
            out_shardings=(sharding,) * n_outs,
        )

    def put_inputs(self, in_maps):
        """in_maps: per-core dict name->np.ndarray. Returns device arrays."""
        jax = self.jax
        sharding = jax.sharding.NamedSharding(self.mesh, self.spec)
        arrs = []
        for name in self.in_names:
            cat = np.concatenate([np.asarray(m[name]) for m in in_maps], axis=0)
            arrs.append(jax.device_put(cat, sharding))
        jax.block_until_ready(arrs)
        return arrs

    def __call__(self, dev_inputs):
        zs = self.zeros_fn()
        self.jax.block_until_ready(zs)
        out = self.fn(*dev_inputs, *zs)
        self.jax.block_until_ready(out)
        return out

    def time_it(self, dev_inputs, reps=10):
        import time as _t

        ts = []
        for _ in range(reps):
            zs = self.zeros_fn()
            self.jax.block_until_ready(zs)
            t0 = _t.perf_counter()
            out = self.fn(*dev_inputs, *zs)
            self.jax.block_until_ready(out)
            ts.append(_t.perf_counter() - t0)
        return ts

    def to_numpy(self, out):
        n = self.n
        return [
            {
                name: np.asarray(out[i]).reshape(n, *self.out_avals[i].shape)[c]
                for i, name in enumerate(self.out_names)
            }
            for c in range(n)
        ]


_RUNNER = None


def _get_runner():
    global _RUNNER
    if _RUNNER is None:
        _RUNNER = Runner(_get_nc(), B)
    return _RUNNER


def _prep_wb(w, b):
    # ws[p, g*9 + kh*3 + kw] = w[2g + p//64, kh, kw, p%64]
    w = np.asarray(w, dtype=np.float32).reshape(G, 2, 9, C)  # (g, dp, tap, c)
    ws = np.ascontiguousarray(w.transpose(1, 3, 0, 2).reshape(128, G * 9))
    b = np.asarray(b, dtype=np.float32).reshape(G, 2, C)
    bs = np.ascontiguousarray(b.transpose(1, 2, 0).reshape(128, G))
    return ws, bs


def _in_maps(inputs):
    x = np.asarray(inputs["x"], dtype=np.float32)
    ws, bs = _prep_wb(inputs["w"], inputs["b"])
    return [{"xs": np.ascontiguousarray(x[i]), "ws": ws, "bs": bs} for i in range(B)]


def kernel(**inputs) -> np.ndarray:
    r = _get_runner()
    dev = r.put_inputs(_in_maps(inputs))
    res = r.to_numpy(r(dev))
    return np.stack([m["ys"] for m in res], axis=0)
